# revision 22
# baseline (speedup 1.0000x reference)
"""Trainium2 Bass kernel for nn_AndAttention.

B=16384 rows; per row: 2-token self-attention over (x1,x2) [D=1024 each],
concat -> h [2048], then 4x (Linear(2048,2048)+ReLU) and Linear(2048,1024).

Sharding: data-parallel over batch across 8 NeuronCores (2048 rows/core),
weights replicated. No collectives.

Layout strategy (per core):
  - Activations live feature-major in SBUF: hT[feature partition, batch free].
  - 2-token softmax == sigmoid of logit differences; dot products via ACT
    Square-with-accumulate and DVE scalar_tensor_tensor-with-accumulate.
  - Attention combine+transpose fused on the PE:
      psum[d, 0:256] = x1c.T @ [diag(a00)|diag(a10)] + x2c.T @ [diag(a01)|diag(a11)]
    which yields y0^T and y1^T directly (feature-major h0).
  - MLP layers: lhsT = pre-transposed bf16 weight tiles (streamed from DRAM,
    4 k-subtiles per DMA for 1KB descriptors), rhs = hT; psum evicted with
    fused ReLU+bias on the scalar engine.
  - Last layer swaps matmul args (lhsT = hT chunk, rhs = W_last^T tiles) so
    psum comes out in natural [batch, out] layout; bias added on DVE from a
    host-replicated bias tile; DMA straight to the output.
  - Matmul operands are bf16 (fp32 PSUM accumulation): bf16 stationary loads
    use the fast-weight-load path and overlap the previous matmul, unlike
    fp32/fp32r whose weight load serializes (~200ns per matmul).
"""

import sys

if "/opt/trn_rl_repo" not in sys.path:
    sys.path.insert(0, "/opt/trn_rl_repo")

import numpy as np
import ml_dtypes

import concourse.bass as bass
import concourse.tile as tile
from concourse import bacc, mybir
from concourse.bass_utils import run_bass_kernel_spmd
from concourse.masks import make_identity

P = 128
D = 1024
D2 = 2048
DOUT = 1024
N_LAYERS = 4
N_CORES = 8
B = 16384
BC = B // N_CORES           # rows per core = 2048
BP = BC                     # single pass over the whole core batch
NB_TILES = BC // P          # 16 b-tiles of 128 rows per core
KT = D2 // P                # 16 k tiles (contraction)
MT = D2 // P                # 16 m tiles (layer out features)
KG = 4                      # k-subtiles per weight DMA
NCHUNK = 512                # matmul moving free dim
NQ = 512                    # last-layer o-half width
QT = DOUT // NQ             # 2

f32 = mybir.dt.float32
bf16 = mybir.dt.bfloat16
NP_BF16 = np.dtype(ml_dtypes.bfloat16)
AF = mybir.ActivationFunctionType
ALU = mybir.AluOpType


def build_graph(debug_stage=None):
    nc = bacc.Bacc("TRN2", target_bir_lowering=False, debug=False,
                   num_devices=N_CORES)

    x1_ext = nc.declare_dram_parameter("x1", [BC, D], f32, isOutput=False)
    x2_ext = nc.declare_dram_parameter("x2", [BC, D], f32, isOutput=False)
    # weight tiles: [l, m, kg, i(128), kk(4), o(128)] bf16 with
    #   wt[l, m, kg, i, kk, o] = Ws[l, m*128+o, (kg*4+kk)*128+i]
    wt_ext = nc.declare_dram_parameter("wt", [N_LAYERS, MT, KT // KG, P, KG, P],
                                       bf16, isOutput=False)
    # last-layer tiles: [k, i(128), o(1024)] bf16 with wlt[k,i,o] = W_last[o, k*128+i]
    wlt_ext = nc.declare_dram_parameter("wlt", [KT, P, DOUT], bf16,
                                        isOutput=False)
    # biases: bst[l, p, m] = bs[l, m*128+p]
    bst_ext = nc.declare_dram_parameter("bst", [N_LAYERS, P, MT], f32,
                                        isOutput=False)
    # b_last replicated across partitions: [128, 1024]
    blb_ext = nc.declare_dram_parameter("blb", [P, DOUT], f32, isOutput=False)
    out_ext = nc.declare_dram_parameter("out", [BC, DOUT], f32, isOutput=True)
    dbg_ext = None
    if debug_stage is not None:
        dbg_ext = nc.declare_dram_parameter("dbg", [P, KT, BP], bf16,
                                            isOutput=True)

    with tile.TileContext(nc) as tc:
        _trace(nc, tc, x1_ext, x2_ext, wt_ext, wlt_ext, bst_ext, blb_ext,
               out_ext, debug_stage, dbg_ext)
    nc.compile()
    return nc


def _trace(nc, tc, x1_ext, x2_ext, wt_ext, wlt_ext, bst_ext, blb_ext, out_ext,
           debug_stage=None, dbg_ext=None):
    from contextlib import ExitStack
    ctx = ExitStack()
    with ctx:
        const = ctx.enter_context(tc.tile_pool(name="const", bufs=1))
        acts = ctx.enter_context(tc.tile_pool(name="acts", bufs=2))
        wpool = ctx.enter_context(tc.tile_pool(name="wpool", bufs=12))
        wlpool = ctx.enter_context(tc.tile_pool(name="wlpool", bufs=1))
        xpool = ctx.enter_context(tc.tile_pool(name="xpool", bufs=3))
        cpool = ctx.enter_context(tc.tile_pool(name="cpool", bufs=2))
        spool = ctx.enter_context(tc.tile_pool(name="spool", bufs=2))
        stpool = ctx.enter_context(tc.tile_pool(name="stpool", bufs=2))
        smpool = ctx.enter_context(tc.tile_pool(name="smpool", bufs=4))
        dpool = ctx.enter_context(tc.tile_pool(name="dpool", bufs=2))
        mpsum = ctx.enter_context(tc.tile_pool(name="mpsum", bufs=8,
                                               space="PSUM"))

        # constants
        ident = const.tile([P, P], f32)
        make_identity(nc, ident)
        bst_sb = const.tile([P, N_LAYERS * MT], f32)
        for l in range(N_LAYERS):
            nc.sync.dma_start(bst_sb[:, l * MT:(l + 1) * MT], bst_ext.ap()[l])
        blb_sb = const.tile([P, DOUT], f32)
        nc.sync.dma_start(blb_sb[:], blb_ext.ap()[:, :])

        # preload all last-layer weight tiles (4MB bf16) once, up front
        wl_tiles = []
        for k in range(KT):
            wl = wlpool.tile([P, DOUT], bf16, name=f"wl{k}")
            nc.sync.dma_start(wl[:], wlt_ext.ap()[k])
            wl_tiles.append(wl)

        # ---------- attention: build h0T [2048 feat, 2048 batch] ----------
        h0 = acts.tile([P, KT, BP], bf16, name="hbuf")

        def attn_tiles(t_lo, t_hi):
            for t in range(t_lo, t_hi):
                x1t = xpool.tile([P, D], f32, name="xt")
                nc.sync.dma_start(x1t[:], x1_ext.ap()[t * P:(t + 1) * P, :])
                x2t = xpool.tile([P, D], f32, name="xt")
                nc.sync.dma_start(x2t[:], x2_ext.ap()[t * P:(t + 1) * P, :])

                stat = smpool.tile([P, 4], f32, name="stat")
                # logits (already include the 1/32 temperature):
                # s11/s22 via ACT Square(x/sqrt(32)) with accumulate,
                # s12 via DVE (x1*(1/32))*x2 with accumulate
                scr = spool.tile([P, D], bf16, name="scr")
                nc.scalar.activation(scr[:], x1t[:], AF.Square,
                                     scale=float(1.0 / np.sqrt(32.0)),
                                     accum_out=stat[:, 0:1])
                scr2 = spool.tile([P, D], bf16, name="scr")
                nc.vector.scalar_tensor_tensor(scr2[:], x1t[:], 1.0 / 32.0,
                                               x2t[:], ALU.mult, ALU.mult,
                                               accum_out=stat[:, 1:2])
                scr3 = spool.tile([P, D], bf16, name="scr")
                nc.scalar.activation(scr3[:], x2t[:], AF.Square,
                                     scale=float(1.0 / np.sqrt(32.0)),
                                     accum_out=stat[:, 2:3])

                dt_ = smpool.tile([P, 2], f32, name="dt")
                nc.vector.tensor_sub(dt_[:, 0:1], stat[:, 0:1], stat[:, 1:2])
                nc.vector.tensor_sub(dt_[:, 1:2], stat[:, 1:2], stat[:, 2:3])

                coef = smpool.tile([P, 4], f32, name="coef")
                nc.scalar.activation(coef[:, 0:1], dt_[:, 0:1], AF.Sigmoid)
                nc.scalar.activation(coef[:, 1:2], dt_[:, 0:1], AF.Sigmoid,
                                     scale=-1.0)
                nc.scalar.activation(coef[:, 2:3], dt_[:, 1:2], AF.Sigmoid)
                nc.scalar.activation(coef[:, 3:4], dt_[:, 1:2], AF.Sigmoid,
                                     scale=-1.0)

                # diagA = [diag(a00)|diag(a10)], diagB = [diag(a01)|diag(a11)]
                # built on the (otherwise idle) gpsimd engine
                diagA = dpool.tile([P, 2 * P], bf16, name="diagA")
                nc.gpsimd.tensor_scalar_mul(diagA[:, 0:P], ident[:],
                                            coef[:, 0:1])
                nc.gpsimd.tensor_scalar_mul(diagA[:, P:2 * P], ident[:],
                                            coef[:, 2:3])
                diagB = dpool.tile([P, 2 * P], bf16, name="diagB")
                nc.gpsimd.tensor_scalar_mul(diagB[:, 0:P], ident[:],
                                            coef[:, 1:2])
                nc.gpsimd.tensor_scalar_mul(diagB[:, P:2 * P], ident[:],
                                            coef[:, 3:4])

                # bf16 copies of x for the PE
                xc1 = cpool.tile([P, D], bf16, name="xc1")
                nc.scalar.copy(xc1[:], x1t[:])
                xc2 = cpool.tile([P, D], bf16, name="xc2")
                nc.scalar.copy(xc2[:], x2t[:])

                col = t * P
                for dc in range(D // P):  # 8 feature chunks
                    ps = mpsum.tile([P, NCHUNK], f32, name="mps")
                    nc.tensor.matmul(ps[:, 0:2 * P],
                                     xc1[:, dc * P:(dc + 1) * P],
                                     diagA[:], start=True, stop=False)
                    nc.tensor.matmul(ps[:, 0:2 * P],
                                     xc2[:, dc * P:(dc + 1) * P],
                                     diagB[:], start=False, stop=True)
                    # one strided eviction writes both token chunks
                    # (features dc and 8+dc); alternate ACT/DVE
                    dst = h0[:, dc::8, col:col + P]
                    if dc % 2 == 0:
                        nc.scalar.copy(dst, ps[:, 0:2 * P])
                    else:
                        nc.vector.tensor_copy(dst, ps[:, 0:2 * P])

        def layer1_half(h_in, h_out, n0):
            for m in range(MT):
                pss = [mpsum.tile([P, NCHUNK], f32, name="mps")
                       for _ in range(2)]
                for kg in range(KT // KG):
                    wt = wpool.tile([P, KG, P], bf16, name="wt")
                    nc.sync.dma_start(wt[:], wt_ext.ap()[0, m, kg])
                    for kk in range(KG):
                        k = kg * KG + kk
                        first = (k == 0)
                        last = (k == KT - 1)
                        for j in range(2):
                            n = n0 + j
                            nc.tensor.matmul(
                                pss[j][:], wt[:, kk, :],
                                h_in[:, k, n * NCHUNK:(n + 1) * NCHUNK],
                                start=first, stop=last)
                bias = bst_sb[:, m:m + 1]
                for j in range(2):
                    n = n0 + j
                    nc.scalar.activation(
                        h_out[:, m, n * NCHUNK:(n + 1) * NCHUNK],
                        pss[j][:], AF.Relu, bias=bias)

        if debug_stage == "attn":
            attn_tiles(0, NB_TILES)
            nc.sync.dma_start(dbg_ext.ap()[:, :, :], h0[:])
            return

        # interleave: layer-1 n-chunk pairs only need a half of the batch
        # columns, so the PE can run layer 1 on the first half while the
        # vector/scalar engines finish attention on the second half.
        attn_tiles(0, NB_TILES // 2)
        h1 = acts.tile([P, KT, BP], bf16, name="hbuf")
        layer1_half(h0, h1, 0)
        attn_tiles(NB_TILES // 2, NB_TILES)
        layer1_half(h0, h1, 2)
        h = h1

        # ---------- MLP layers 2..4 (feature-major) ----------
        for l in range(1, N_LAYERS):
            hout = acts.tile([P, KT, BP], bf16, name="hbuf")
            for m in range(MT):
                pss = [mpsum.tile([P, NCHUNK], f32, name="mps")
                       for _ in range(BP // NCHUNK)]
                for kg in range(KT // KG):
                    wt = wpool.tile([P, KG, P], bf16, name="wt")
                    nc.sync.dma_start(wt[:], wt_ext.ap()[l, m, kg])
                    for kk in range(KG):
                        k = kg * KG + kk
                        first = (k == 0)
                        last = (k == KT - 1)
                        for n in range(BP // NCHUNK):
                            nc.tensor.matmul(
                                pss[n][:], wt[:, kk, :],
                                h[:, k, n * NCHUNK:(n + 1) * NCHUNK],
                                start=first, stop=last)
                bias = bst_sb[:, l * MT + m:l * MT + m + 1]
                for n in range(BP // NCHUNK):
                    nc.scalar.activation(hout[:, m, n * NCHUNK:(n + 1) * NCHUNK],
                                         pss[n][:], AF.Relu, bias=bias)
            h = hout

        if debug_stage == "mlp":
            nc.sync.dma_start(dbg_ext.ap()[:, :, :], h[:])
            return

        # ---------- last layer: natural-layout output ----------
        for m in range(BP // P):  # 16 batch chunks of 128
            pss = [mpsum.tile([P, NCHUNK], f32, name="mps")
                   for _ in range(QT)]
            for k in range(KT):
                for q in range(QT):
                    nc.tensor.matmul(pss[q][:], h[:, k, m * P:(m + 1) * P],
                                     wl_tiles[k][:, q * NQ:(q + 1) * NQ],
                                     start=(k == 0), stop=(k == KT - 1))
            for q in range(QT):
                stg = stpool.tile([P, NQ], f32, name="stg")
                nc.vector.tensor_add(stg[:], pss[q][:],
                                     blb_sb[:, q * NQ:(q + 1) * NQ])
                r0 = m * P
                nc.sync.dma_start(
                    out_ext.ap()[r0:r0 + P, q * NQ:(q + 1) * NQ], stg[:])


def prep_inputs(x1, x2, Ws, bs, W_last, b_last):
    """Host-side layout prep shared by all cores (weights) + per-core shards."""
    wt = np.ascontiguousarray(
        Ws.reshape(N_LAYERS, MT, P, KT // KG, KG, P)
        .transpose(0, 1, 3, 5, 4, 2)).astype(NP_BF16)
    wlt = np.ascontiguousarray(
        W_last.reshape(DOUT, KT, P).transpose(1, 2, 0)).astype(NP_BF16)
    bst = np.ascontiguousarray(
        bs.reshape(N_LAYERS, MT, P).transpose(0, 2, 1))
    blb = np.ascontiguousarray(np.broadcast_to(b_last, (P, DOUT)))
    shared = {"wt": wt, "wlt": wlt, "bst": bst, "blb": blb}
    in_maps = []
    for c in range(N_CORES):
        sl = slice(c * BC, (c + 1) * BC)
        m = {"x1": np.ascontiguousarray(x1[sl]),
             "x2": np.ascontiguousarray(x2[sl])}
        m.update(shared)
        in_maps.append(m)
    return in_maps


_compiled_nc = None


def kernel(x1, x2, Ws, bs, W_last, b_last):
    global _compiled_nc
    x1 = np.asarray(x1, dtype=np.float32)
    x2 = np.asarray(x2, dtype=np.float32)
    Ws = np.asarray(Ws, dtype=np.float32)
    bs = np.asarray(bs, dtype=np.float32)
    W_last = np.asarray(W_last, dtype=np.float32)
    b_last = np.asarray(b_last, dtype=np.float32)

    if _compiled_nc is None:
        _compiled_nc = build_graph()
    in_maps = prep_inputs(x1, x2, Ws, bs, W_last, b_last)
    res = run_bass_kernel_spmd(_compiled_nc, in_maps,
                               core_ids=list(range(N_CORES)))
    out = np.concatenate([res.results[c]["out"] for c in range(N_CORES)],
                         axis=0)
    return out.astype(np.float32)


# revision 24
# speedup vs baseline: 1.3004x; 1.3004x over previous
"""Trainium2 Bass kernel for nn_AndAttention.

B=16384 rows; per row: 2-token self-attention over (x1,x2) [D=1024 each],
concat -> h [2048], then 4x (Linear(2048,2048)+ReLU) and Linear(2048,1024).

Sharding: data-parallel over batch across 8 NeuronCores (2048 rows/core),
weights replicated. No collectives.

Layout strategy (per core):
  - Activations live feature-major in SBUF: hT[feature partition, batch free].
  - 2-token softmax == sigmoid of logit differences; dot products via ACT
    Square-with-accumulate and DVE scalar_tensor_tensor-with-accumulate.
  - Attention combine+transpose fused on the PE:
      psum[d, 0:256] = x1c.T @ [diag(a00)|diag(a10)] + x2c.T @ [diag(a01)|diag(a11)]
    which yields y0^T and y1^T directly (feature-major h0).
  - MLP layers: lhsT = pre-transposed bf16 weight tiles (streamed from DRAM,
    4 k-subtiles per DMA for 1KB descriptors), rhs = hT; psum evicted with
    fused ReLU+bias on the scalar engine.
  - Last layer swaps matmul args (lhsT = hT chunk, rhs = W_last^T tiles) so
    psum comes out in natural [batch, out] layout; bias added on DVE from a
    host-replicated bias tile; DMA straight to the output.
  - Matmul operands are bf16 (fp32 PSUM accumulation): bf16 stationary loads
    use the fast-weight-load path and overlap the previous matmul, unlike
    fp32/fp32r whose weight load serializes (~200ns per matmul).
"""

import sys

if "/opt/trn_rl_repo" not in sys.path:
    sys.path.insert(0, "/opt/trn_rl_repo")

import numpy as np
import ml_dtypes

import concourse.bass as bass
import concourse.tile as tile
from concourse import bacc, mybir
from concourse.bass_utils import run_bass_kernel_spmd
from concourse.masks import make_identity

P = 128
D = 1024
D2 = 2048
DOUT = 1024
N_LAYERS = 4
N_CORES = 8
B = 16384
BC = B // N_CORES           # rows per core = 2048
BP = BC                     # single pass over the whole core batch
NB_TILES = BC // P          # 16 b-tiles of 128 rows per core
KT = D2 // P                # 16 k tiles (contraction)
MT = D2 // P                # 16 m tiles (layer out features)
KG = 4                      # k-subtiles per weight DMA
NCHUNK = 512                # matmul moving free dim
NQ = 512                    # last-layer o-half width
QT = DOUT // NQ             # 2

f32 = mybir.dt.float32
bf16 = mybir.dt.bfloat16
NP_BF16 = np.dtype(ml_dtypes.bfloat16)
AF = mybir.ActivationFunctionType
ALU = mybir.AluOpType


def build_graph(debug_stage=None):
    nc = bacc.Bacc("TRN2", target_bir_lowering=False, debug=False,
                   num_devices=N_CORES)

    x1_ext = nc.declare_dram_parameter("x1", [BC, D], f32, isOutput=False)
    x2_ext = nc.declare_dram_parameter("x2", [BC, D], f32, isOutput=False)
    # weight tiles: [l, m, kg, i(128), kk(4), o(128)] bf16 with
    #   wt[l, m, kg, i, kk, o] = Ws[l, m*128+o, (kg*4+kk)*128+i]
    wt_ext = nc.declare_dram_parameter("wt", [N_LAYERS, MT, KT // KG, P, KG, P],
                                       bf16, isOutput=False)
    # last-layer tiles: [k, i(128), o(1024)] bf16 with wlt[k,i,o] = W_last[o, k*128+i]
    wlt_ext = nc.declare_dram_parameter("wlt", [KT, P, DOUT], bf16,
                                        isOutput=False)
    # biases: bst[l, p, m] = bs[l, m*128+p]
    bst_ext = nc.declare_dram_parameter("bst", [N_LAYERS, P, MT], f32,
                                        isOutput=False)
    # b_last replicated across partitions: [128, 1024]
    blb_ext = nc.declare_dram_parameter("blb", [P, DOUT], f32, isOutput=False)
    out_ext = nc.declare_dram_parameter("out", [BC, DOUT], f32, isOutput=True)
    dbg_ext = None
    if debug_stage is not None:
        dbg_ext = nc.declare_dram_parameter("dbg", [P, KT, BP], bf16,
                                            isOutput=True)

    with tile.TileContext(nc) as tc:
        _trace(nc, tc, x1_ext, x2_ext, wt_ext, wlt_ext, bst_ext, blb_ext,
               out_ext, debug_stage, dbg_ext)
    nc.compile()
    return nc


def _trace(nc, tc, x1_ext, x2_ext, wt_ext, wlt_ext, bst_ext, blb_ext, out_ext,
           debug_stage=None, dbg_ext=None):
    from contextlib import ExitStack
    ctx = ExitStack()
    with ctx:
        const = ctx.enter_context(tc.tile_pool(name="const", bufs=1))
        acts = ctx.enter_context(tc.tile_pool(name="acts", bufs=2))
        wpool = ctx.enter_context(tc.tile_pool(name="wpool", bufs=12))
        wlpool = ctx.enter_context(tc.tile_pool(name="wlpool", bufs=1))
        xpool = ctx.enter_context(tc.tile_pool(name="xpool", bufs=3))
        cpool = ctx.enter_context(tc.tile_pool(name="cpool", bufs=2))
        spool = ctx.enter_context(tc.tile_pool(name="spool", bufs=2))
        stpool = ctx.enter_context(tc.tile_pool(name="stpool", bufs=2))
        smpool = ctx.enter_context(tc.tile_pool(name="smpool", bufs=4))
        dpool = ctx.enter_context(tc.tile_pool(name="dpool", bufs=2))
        mpsum = ctx.enter_context(tc.tile_pool(name="mpsum", bufs=8,
                                               space="PSUM"))

        # constants
        ident = const.tile([P, P], f32)
        make_identity(nc, ident)
        bst_sb = const.tile([P, N_LAYERS * MT], f32)
        for l in range(N_LAYERS):
            nc.sync.dma_start(bst_sb[:, l * MT:(l + 1) * MT], bst_ext.ap()[l])
        blb_sb = const.tile([P, DOUT], f32)
        nc.sync.dma_start(blb_sb[:], blb_ext.ap()[:, :])

        # preload all last-layer weight tiles (4MB bf16) once, up front
        wl_tiles = []
        for k in range(KT):
            wl = wlpool.tile([P, DOUT], bf16, name=f"wl{k}")
            nc.sync.dma_start(wl[:], wlt_ext.ap()[k])
            wl_tiles.append(wl)

        # ---------- attention: build h0T [2048 feat, 2048 batch] ----------
        h0 = acts.tile([P, KT, BP], bf16, name="hbuf")

        def attn_tiles(t_lo, t_hi):
            for t in range(t_lo, t_hi):
                x1t = xpool.tile([P, D], f32, name="xt")
                nc.sync.dma_start(x1t[:], x1_ext.ap()[t * P:(t + 1) * P, :])
                x2t = xpool.tile([P, D], f32, name="xt")
                nc.sync.dma_start(x2t[:], x2_ext.ap()[t * P:(t + 1) * P, :])

                stat = smpool.tile([P, 4], f32, name="stat")
                # logits (already include the 1/32 temperature):
                # s11/s22 via ACT Square(x/sqrt(32)) with accumulate,
                # s12 via DVE (x1*(1/32))*x2 with accumulate
                scr = spool.tile([P, D], bf16, name="scr")
                nc.scalar.activation(scr[:], x1t[:], AF.Square,
                                     scale=float(1.0 / np.sqrt(32.0)),
                                     accum_out=stat[:, 0:1])
                scr2 = spool.tile([P, D], bf16, name="scr")
                nc.vector.scalar_tensor_tensor(scr2[:], x1t[:], 1.0 / 32.0,
                                               x2t[:], ALU.mult, ALU.mult,
                                               accum_out=stat[:, 1:2])
                scr3 = spool.tile([P, D], bf16, name="scr")
                nc.scalar.activation(scr3[:], x2t[:], AF.Square,
                                     scale=float(1.0 / np.sqrt(32.0)),
                                     accum_out=stat[:, 2:3])

                dt_ = smpool.tile([P, 2], f32, name="dt")
                nc.vector.tensor_sub(dt_[:, 0:1], stat[:, 0:1], stat[:, 1:2])
                nc.vector.tensor_sub(dt_[:, 1:2], stat[:, 1:2], stat[:, 2:3])

                coef = smpool.tile([P, 4], f32, name="coef")
                nc.scalar.activation(coef[:, 0:1], dt_[:, 0:1], AF.Sigmoid)
                nc.scalar.activation(coef[:, 1:2], dt_[:, 0:1], AF.Sigmoid,
                                     scale=-1.0)
                nc.scalar.activation(coef[:, 2:3], dt_[:, 1:2], AF.Sigmoid)
                nc.scalar.activation(coef[:, 3:4], dt_[:, 1:2], AF.Sigmoid,
                                     scale=-1.0)

                # diagA = [diag(a00)|diag(a10)], diagB = [diag(a01)|diag(a11)]
                diagA = dpool.tile([P, 2 * P], bf16, name="diagA")
                nc.vector.tensor_scalar_mul(diagA[:, 0:P], ident[:],
                                            coef[:, 0:1])
                nc.vector.tensor_scalar_mul(diagA[:, P:2 * P], ident[:],
                                            coef[:, 2:3])
                diagB = dpool.tile([P, 2 * P], bf16, name="diagB")
                nc.vector.tensor_scalar_mul(diagB[:, 0:P], ident[:],
                                            coef[:, 1:2])
                nc.vector.tensor_scalar_mul(diagB[:, P:2 * P], ident[:],
                                            coef[:, 3:4])

                # bf16 copies of x for the PE via casting DMA (gpsimd-
                # initiated DMAs may cast; engines stay free)
                xc1 = cpool.tile([P, D], bf16, name="xc1")
                nc.gpsimd.dma_start(xc1[:], x1_ext.ap()[t * P:(t + 1) * P, :])
                xc2 = cpool.tile([P, D], bf16, name="xc2")
                nc.gpsimd.dma_start(xc2[:], x2_ext.ap()[t * P:(t + 1) * P, :])

                col = t * P
                for dc in range(D // P):  # 8 feature chunks
                    ps = mpsum.tile([P, NCHUNK], f32, name="mps")
                    nc.tensor.matmul(ps[:, 0:2 * P],
                                     xc1[:, dc * P:(dc + 1) * P],
                                     diagA[:], start=True, stop=False)
                    nc.tensor.matmul(ps[:, 0:2 * P],
                                     xc2[:, dc * P:(dc + 1) * P],
                                     diagB[:], start=False, stop=True)
                    # evictions: token0 chunk on ACT, token1 chunk on DVE
                    nc.scalar.copy(h0[:, dc, col:col + P], ps[:, 0:P])
                    nc.vector.tensor_copy(h0[:, 8 + dc, col:col + P],
                                          ps[:, P:2 * P])

        def layer1_block(h_in, h_out, n):
            for m in range(MT):
                ps = mpsum.tile([P, NCHUNK], f32, name="mps")
                for kg in range(KT // KG):
                    wt = wpool.tile([P, KG, P], bf16, name="wt")
                    nc.sync.dma_start(wt[:], wt_ext.ap()[0, m, kg])
                    for kk in range(KG):
                        k = kg * KG + kk
                        nc.tensor.matmul(
                            ps[:], wt[:, kk, :],
                            h_in[:, k, n * NCHUNK:(n + 1) * NCHUNK],
                            start=(k == 0), stop=(k == KT - 1))
                nc.scalar.activation(h_out[:, m, n * NCHUNK:(n + 1) * NCHUNK],
                                     ps[:], AF.Relu, bias=bst_sb[:, m:m + 1])

        if debug_stage == "attn":
            attn_tiles(0, NB_TILES)
            nc.sync.dma_start(dbg_ext.ap()[:, :, :], h0[:])
            return

        # interleave: each layer-1 n-chunk only needs a quarter of the batch
        # columns, so the PE runs layer 1 on finished quarters while the
        # vector/scalar engines compute attention for the next quarter.
        attn_tiles(0, 4)
        h1 = acts.tile([P, KT, BP], bf16, name="hbuf")
        for n in range(4):
            if n < 3:
                attn_tiles(4 * (n + 1), 4 * (n + 2))
            layer1_block(h0, h1, n)
        h = h1

        # ---------- MLP layers 2..4 (feature-major) ----------
        for l in range(1, N_LAYERS):
            hout = acts.tile([P, KT, BP], bf16, name="hbuf")
            for m in range(MT):
                pss = [mpsum.tile([P, NCHUNK], f32, name="mps")
                       for _ in range(BP // NCHUNK)]
                for kg in range(KT // KG):
                    wt = wpool.tile([P, KG, P], bf16, name="wt")
                    nc.sync.dma_start(wt[:], wt_ext.ap()[l, m, kg])
                    for kk in range(KG):
                        k = kg * KG + kk
                        first = (k == 0)
                        last = (k == KT - 1)
                        for n in range(BP // NCHUNK):
                            nc.tensor.matmul(
                                pss[n][:], wt[:, kk, :],
                                h[:, k, n * NCHUNK:(n + 1) * NCHUNK],
                                start=first, stop=last)
                bias = bst_sb[:, l * MT + m:l * MT + m + 1]
                for n in range(BP // NCHUNK):
                    nc.scalar.activation(hout[:, m, n * NCHUNK:(n + 1) * NCHUNK],
                                         pss[n][:], AF.Relu, bias=bias)
            h = hout

        if debug_stage == "mlp":
            nc.sync.dma_start(dbg_ext.ap()[:, :, :], h[:])
            return

        # ---------- last layer: natural-layout output ----------
        for m in range(BP // P):  # 16 batch chunks of 128
            pss = [mpsum.tile([P, NCHUNK], f32, name="mps")
                   for _ in range(QT)]
            for k in range(KT):
                for q in range(QT):
                    nc.tensor.matmul(pss[q][:], h[:, k, m * P:(m + 1) * P],
                                     wl_tiles[k][:, q * NQ:(q + 1) * NQ],
                                     start=(k == 0), stop=(k == KT - 1))
            for q in range(QT):
                stg = stpool.tile([P, NQ], f32, name="stg")
                nc.vector.tensor_add(stg[:], pss[q][:],
                                     blb_sb[:, q * NQ:(q + 1) * NQ])
                r0 = m * P
                nc.sync.dma_start(
                    out_ext.ap()[r0:r0 + P, q * NQ:(q + 1) * NQ], stg[:])


def prep_inputs(x1, x2, Ws, bs, W_last, b_last):
    """Host-side layout prep shared by all cores (weights) + per-core shards."""
    wt = np.ascontiguousarray(
        Ws.reshape(N_LAYERS, MT, P, KT // KG, KG, P)
        .transpose(0, 1, 3, 5, 4, 2)).astype(NP_BF16)
    wlt = np.ascontiguousarray(
        W_last.reshape(DOUT, KT, P).transpose(1, 2, 0)).astype(NP_BF16)
    bst = np.ascontiguousarray(
        bs.reshape(N_LAYERS, MT, P).transpose(0, 2, 1))
    blb = np.ascontiguousarray(np.broadcast_to(b_last, (P, DOUT)))
    shared = {"wt": wt, "wlt": wlt, "bst": bst, "blb": blb}
    in_maps = []
    for c in range(N_CORES):
        sl = slice(c * BC, (c + 1) * BC)
        m = {"x1": np.ascontiguousarray(x1[sl]),
             "x2": np.ascontiguousarray(x2[sl])}
        m.update(shared)
        in_maps.append(m)
    return in_maps


_compiled_nc = None


def kernel(x1, x2, Ws, bs, W_last, b_last):
    global _compiled_nc
    x1 = np.asarray(x1, dtype=np.float32)
    x2 = np.asarray(x2, dtype=np.float32)
    Ws = np.asarray(Ws, dtype=np.float32)
    bs = np.asarray(bs, dtype=np.float32)
    W_last = np.asarray(W_last, dtype=np.float32)
    b_last = np.asarray(b_last, dtype=np.float32)

    if _compiled_nc is None:
        _compiled_nc = build_graph()
    in_maps = prep_inputs(x1, x2, Ws, bs, W_last, b_last)
    res = run_bass_kernel_spmd(_compiled_nc, in_maps,
                               core_ids=list(range(N_CORES)))
    out = np.concatenate([res.results[c]["out"] for c in range(N_CORES)],
                         axis=0)
    return out.astype(np.float32)


# revision 25
# speedup vs baseline: 1.3174x; 1.0131x over previous
"""Trainium2 Bass kernel for nn_AndAttention.

B=16384 rows; per row: 2-token self-attention over (x1,x2) [D=1024 each],
concat -> h [2048], then 4x (Linear(2048,2048)+ReLU) and Linear(2048,1024).

Sharding: data-parallel over batch across 8 NeuronCores (2048 rows/core),
weights replicated. No collectives.

Layout strategy (per core):
  - Activations live feature-major in SBUF: hT[feature partition, batch free].
  - 2-token softmax == sigmoid of logit differences; dot products via ACT
    Square-with-accumulate and DVE scalar_tensor_tensor-with-accumulate.
  - Attention combine+transpose fused on the PE:
      psum[d, 0:256] = x1c.T @ [diag(a00)|diag(a10)] + x2c.T @ [diag(a01)|diag(a11)]
    which yields y0^T and y1^T directly (feature-major h0).
  - MLP layers: lhsT = pre-transposed bf16 weight tiles (streamed from DRAM,
    4 k-subtiles per DMA for 1KB descriptors), rhs = hT; psum evicted with
    fused ReLU+bias on the scalar engine.
  - Last layer swaps matmul args (lhsT = hT chunk, rhs = W_last^T tiles) so
    psum comes out in natural [batch, out] layout; bias added on DVE from a
    host-replicated bias tile; DMA straight to the output.
  - Matmul operands are bf16 (fp32 PSUM accumulation): bf16 stationary loads
    use the fast-weight-load path and overlap the previous matmul, unlike
    fp32/fp32r whose weight load serializes (~200ns per matmul).
"""

import sys

if "/opt/trn_rl_repo" not in sys.path:
    sys.path.insert(0, "/opt/trn_rl_repo")

import numpy as np
import ml_dtypes

import concourse.bass as bass
import concourse.tile as tile
from concourse import bacc, mybir
from concourse.bass_utils import run_bass_kernel_spmd
from concourse.masks import make_identity

P = 128
D = 1024
D2 = 2048
DOUT = 1024
N_LAYERS = 4
N_CORES = 8
B = 16384
BC = B // N_CORES           # rows per core = 2048
BP = BC                     # single pass over the whole core batch
NB_TILES = BC // P          # 16 b-tiles of 128 rows per core
KT = D2 // P                # 16 k tiles (contraction)
MT = D2 // P                # 16 m tiles (layer out features)
KG = 4                      # k-subtiles per weight DMA
NCHUNK = 512                # matmul moving free dim
NQ = 512                    # last-layer o-half width
QT = DOUT // NQ             # 2

f32 = mybir.dt.float32
bf16 = mybir.dt.bfloat16
NP_BF16 = np.dtype(ml_dtypes.bfloat16)
AF = mybir.ActivationFunctionType
ALU = mybir.AluOpType


def build_graph(debug_stage=None):
    nc = bacc.Bacc("TRN2", target_bir_lowering=False, debug=False,
                   num_devices=N_CORES)

    x1_ext = nc.declare_dram_parameter("x1", [BC, D], f32, isOutput=False)
    x2_ext = nc.declare_dram_parameter("x2", [BC, D], f32, isOutput=False)
    # weight tiles: [l, m, kg, i(128), kk(4), o(128)] bf16 with
    #   wt[l, m, kg, i, kk, o] = Ws[l, m*128+o, (kg*4+kk)*128+i]
    wt_ext = nc.declare_dram_parameter("wt", [N_LAYERS, MT, KT // KG, P, KG, P],
                                       bf16, isOutput=False)
    # last-layer tiles: [k, i(128), o(1024)] bf16 with wlt[k,i,o] = W_last[o, k*128+i]
    wlt_ext = nc.declare_dram_parameter("wlt", [KT, P, DOUT], bf16,
                                        isOutput=False)
    # biases: bst[l, p, m] = bs[l, m*128+p]
    bst_ext = nc.declare_dram_parameter("bst", [N_LAYERS, P, MT], f32,
                                        isOutput=False)
    # b_last replicated across partitions: [128, 1024]
    blb_ext = nc.declare_dram_parameter("blb", [P, DOUT], f32, isOutput=False)
    out_ext = nc.declare_dram_parameter("out", [BC, DOUT], f32, isOutput=True)
    dbg_ext = None
    if debug_stage is not None:
        dbg_ext = nc.declare_dram_parameter("dbg", [P, KT, BP], bf16,
                                            isOutput=True)

    with tile.TileContext(nc) as tc:
        _trace(nc, tc, x1_ext, x2_ext, wt_ext, wlt_ext, bst_ext, blb_ext,
               out_ext, debug_stage, dbg_ext)
    nc.compile()
    return nc


def _trace(nc, tc, x1_ext, x2_ext, wt_ext, wlt_ext, bst_ext, blb_ext, out_ext,
           debug_stage=None, dbg_ext=None):
    from contextlib import ExitStack
    ctx = ExitStack()
    with ctx:
        const = ctx.enter_context(tc.tile_pool(name="const", bufs=1))
        acts = ctx.enter_context(tc.tile_pool(name="acts", bufs=2))
        wpool = ctx.enter_context(tc.tile_pool(name="wpool", bufs=16))
        wlpool = ctx.enter_context(tc.tile_pool(name="wlpool", bufs=1))
        cpool = ctx.enter_context(tc.tile_pool(name="cpool", bufs=3))
        spool = ctx.enter_context(tc.tile_pool(name="spool", bufs=2))
        stpool = ctx.enter_context(tc.tile_pool(name="stpool", bufs=2))
        smpool = ctx.enter_context(tc.tile_pool(name="smpool", bufs=4))
        dpool = ctx.enter_context(tc.tile_pool(name="dpool", bufs=2))
        mpsum = ctx.enter_context(tc.tile_pool(name="mpsum", bufs=8,
                                               space="PSUM"))

        # constants
        ident = const.tile([P, P], f32)
        make_identity(nc, ident)
        bst_sb = const.tile([P, N_LAYERS * MT], f32)
        for l in range(N_LAYERS):
            nc.sync.dma_start(bst_sb[:, l * MT:(l + 1) * MT], bst_ext.ap()[l])
        blb_sb = const.tile([P, DOUT], f32)
        nc.sync.dma_start(blb_sb[:], blb_ext.ap()[:, :])

        # preload all last-layer weight tiles (4MB bf16) once, up front
        wl_tiles = []
        for k in range(KT):
            wl = wlpool.tile([P, DOUT], bf16, name=f"wl{k}")
            nc.sync.dma_start(wl[:], wlt_ext.ap()[k])
            wl_tiles.append(wl)

        # ---------- attention: build h0T [2048 feat, 2048 batch] ----------
        h0 = acts.tile([P, KT, BP], bf16, name="hbuf")

        def attn_tiles(t_lo, t_hi):
            for t in range(t_lo, t_hi):
                # bf16 copies of x via casting DMA (gpsimd-initiated DMAs may
                # cast); used for both the dot products and the PE combine
                xc1 = cpool.tile([P, D], bf16, name="xc1")
                nc.gpsimd.dma_start(xc1[:], x1_ext.ap()[t * P:(t + 1) * P, :])
                xc2 = cpool.tile([P, D], bf16, name="xc2")
                nc.gpsimd.dma_start(xc2[:], x2_ext.ap()[t * P:(t + 1) * P, :])

                stat = smpool.tile([P, 4], f32, name="stat")
                # logits (already include the 1/32 temperature):
                # s11/s22 via ACT Square(x/sqrt(32)) with accumulate,
                # s12 via DVE (x1*(1/32))*x2 with accumulate
                scr = spool.tile([P, D], bf16, name="scr")
                nc.scalar.activation(scr[:], xc1[:], AF.Square,
                                     scale=float(1.0 / np.sqrt(32.0)),
                                     accum_out=stat[:, 0:1])
                scr2 = spool.tile([P, D], bf16, name="scr")
                nc.vector.scalar_tensor_tensor(scr2[:], xc1[:], 1.0 / 32.0,
                                               xc2[:], ALU.mult, ALU.mult,
                                               accum_out=stat[:, 1:2])
                scr3 = spool.tile([P, D], bf16, name="scr")
                nc.scalar.activation(scr3[:], xc2[:], AF.Square,
                                     scale=float(1.0 / np.sqrt(32.0)),
                                     accum_out=stat[:, 2:3])

                dt_ = smpool.tile([P, 2], f32, name="dt")
                nc.vector.tensor_sub(dt_[:, 0:1], stat[:, 0:1], stat[:, 1:2])
                nc.vector.tensor_sub(dt_[:, 1:2], stat[:, 1:2], stat[:, 2:3])

                coef = smpool.tile([P, 4], f32, name="coef")
                nc.scalar.activation(coef[:, 0:1], dt_[:, 0:1], AF.Sigmoid)
                nc.scalar.activation(coef[:, 1:2], dt_[:, 0:1], AF.Sigmoid,
                                     scale=-1.0)
                nc.scalar.activation(coef[:, 2:3], dt_[:, 1:2], AF.Sigmoid)
                nc.scalar.activation(coef[:, 3:4], dt_[:, 1:2], AF.Sigmoid,
                                     scale=-1.0)

                # diagA = [diag(a00)|diag(a10)], diagB = [diag(a01)|diag(a11)]
                diagA = dpool.tile([P, 2 * P], bf16, name="diagA")
                nc.vector.tensor_scalar_mul(diagA[:, 0:P], ident[:],
                                            coef[:, 0:1])
                nc.vector.tensor_scalar_mul(diagA[:, P:2 * P], ident[:],
                                            coef[:, 2:3])
                diagB = dpool.tile([P, 2 * P], bf16, name="diagB")
                nc.vector.tensor_scalar_mul(diagB[:, 0:P], ident[:],
                                            coef[:, 1:2])
                nc.vector.tensor_scalar_mul(diagB[:, P:2 * P], ident[:],
                                            coef[:, 3:4])

                col = t * P
                for dc in range(D // P):  # 8 feature chunks
                    ps = mpsum.tile([P, NCHUNK], f32, name="mps")
                    nc.tensor.matmul(ps[:, 0:2 * P],
                                     xc1[:, dc * P:(dc + 1) * P],
                                     diagA[:], start=True, stop=False)
                    nc.tensor.matmul(ps[:, 0:2 * P],
                                     xc2[:, dc * P:(dc + 1) * P],
                                     diagB[:], start=False, stop=True)
                    # evictions: token0 chunk on ACT, token1 chunk on DVE
                    nc.scalar.copy(h0[:, dc, col:col + P], ps[:, 0:P])
                    nc.vector.tensor_copy(h0[:, 8 + dc, col:col + P],
                                          ps[:, P:2 * P])

        def layer1_block(h_in, h_out, n):
            for m in range(MT):
                ps = mpsum.tile([P, NCHUNK], f32, name="mps")
                for kg in range(KT // KG):
                    wt = wpool.tile([P, KG, P], bf16, name="wt")
                    nc.sync.dma_start(wt[:], wt_ext.ap()[0, m, kg])
                    for kk in range(KG):
                        k = kg * KG + kk
                        nc.tensor.matmul(
                            ps[:], wt[:, kk, :],
                            h_in[:, k, n * NCHUNK:(n + 1) * NCHUNK],
                            start=(k == 0), stop=(k == KT - 1))
                nc.scalar.activation(h_out[:, m, n * NCHUNK:(n + 1) * NCHUNK],
                                     ps[:], AF.Relu, bias=bst_sb[:, m:m + 1])

        if debug_stage == "attn":
            attn_tiles(0, NB_TILES)
            nc.sync.dma_start(dbg_ext.ap()[:, :, :], h0[:])
            return

        # interleave: each layer-1 n-chunk only needs a quarter of the batch
        # columns, so the PE runs layer 1 on finished quarters while the
        # vector/scalar engines compute attention for the next quarter.
        attn_tiles(0, 4)
        h1 = acts.tile([P, KT, BP], bf16, name="hbuf")
        for n in range(4):
            if n < 3:
                attn_tiles(4 * (n + 1), 4 * (n + 2))
            layer1_block(h0, h1, n)
        h = h1

        # ---------- MLP layers 2..4 (feature-major) ----------
        for l in range(1, N_LAYERS):
            hout = acts.tile([P, KT, BP], bf16, name="hbuf")
            for m in range(MT):
                pss = [mpsum.tile([P, NCHUNK], f32, name="mps")
                       for _ in range(BP // NCHUNK)]
                for kg in range(KT // KG):
                    wt = wpool.tile([P, KG, P], bf16, name="wt")
                    nc.sync.dma_start(wt[:], wt_ext.ap()[l, m, kg])
                    for kk in range(KG):
                        k = kg * KG + kk
                        first = (k == 0)
                        last = (k == KT - 1)
                        for n in range(BP // NCHUNK):
                            nc.tensor.matmul(
                                pss[n][:], wt[:, kk, :],
                                h[:, k, n * NCHUNK:(n + 1) * NCHUNK],
                                start=first, stop=last)
                bias = bst_sb[:, l * MT + m:l * MT + m + 1]
                for n in range(BP // NCHUNK):
                    nc.scalar.activation(hout[:, m, n * NCHUNK:(n + 1) * NCHUNK],
                                         pss[n][:], AF.Relu, bias=bias)
            h = hout

        if debug_stage == "mlp":
            nc.sync.dma_start(dbg_ext.ap()[:, :, :], h[:])
            return

        # ---------- last layer: natural-layout output ----------
        for m in range(BP // P):  # 16 batch chunks of 128
            pss = [mpsum.tile([P, NCHUNK], f32, name="mps")
                   for _ in range(QT)]
            for k in range(KT):
                for q in range(QT):
                    nc.tensor.matmul(pss[q][:], h[:, k, m * P:(m + 1) * P],
                                     wl_tiles[k][:, q * NQ:(q + 1) * NQ],
                                     start=(k == 0), stop=(k == KT - 1))
            for q in range(QT):
                stg = stpool.tile([P, NQ], f32, name="stg")
                nc.vector.tensor_add(stg[:], pss[q][:],
                                     blb_sb[:, q * NQ:(q + 1) * NQ])
                r0 = m * P
                nc.sync.dma_start(
                    out_ext.ap()[r0:r0 + P, q * NQ:(q + 1) * NQ], stg[:])


def prep_inputs(x1, x2, Ws, bs, W_last, b_last):
    """Host-side layout prep shared by all cores (weights) + per-core shards."""
    wt = np.ascontiguousarray(
        Ws.reshape(N_LAYERS, MT, P, KT // KG, KG, P)
        .transpose(0, 1, 3, 5, 4, 2)).astype(NP_BF16)
    wlt = np.ascontiguousarray(
        W_last.reshape(DOUT, KT, P).transpose(1, 2, 0)).astype(NP_BF16)
    bst = np.ascontiguousarray(
        bs.reshape(N_LAYERS, MT, P).transpose(0, 2, 1))
    blb = np.ascontiguousarray(np.broadcast_to(b_last, (P, DOUT)))
    shared = {"wt": wt, "wlt": wlt, "bst": bst, "blb": blb}
    in_maps = []
    for c in range(N_CORES):
        sl = slice(c * BC, (c + 1) * BC)
        m = {"x1": np.ascontiguousarray(x1[sl]),
             "x2": np.ascontiguousarray(x2[sl])}
        m.update(shared)
        in_maps.append(m)
    return in_maps


_compiled_nc = None


def kernel(x1, x2, Ws, bs, W_last, b_last):
    global _compiled_nc
    x1 = np.asarray(x1, dtype=np.float32)
    x2 = np.asarray(x2, dtype=np.float32)
    Ws = np.asarray(Ws, dtype=np.float32)
    bs = np.asarray(bs, dtype=np.float32)
    W_last = np.asarray(W_last, dtype=np.float32)
    b_last = np.asarray(b_last, dtype=np.float32)

    if _compiled_nc is None:
        _compiled_nc = build_graph()
    in_maps = prep_inputs(x1, x2, Ws, bs, W_last, b_last)
    res = run_bass_kernel_spmd(_compiled_nc, in_maps,
                               core_ids=list(range(N_CORES)))
    out = np.concatenate([res.results[c]["out"] for c in range(N_CORES)],
                         axis=0)
    return out.astype(np.float32)
